# revision 13
# baseline (speedup 1.0000x reference)
"""DetectorLoss on 8 Trainium2 NeuronCores (Bass/Tile).

Strategy (data-parallel over batch, 4 images per core):
  * Sparse positive-cell reads use the HW indirect-DMA shape (ONE offset
    per partition, contiguous block): the host interleaves
    [d0,d1,d2,d3,obj] per cell so 4 gathers cover box deltas AND obj;
    4 more cover cls.  Gathers run on a clean Pool queue (input DMAs are
    dispatched from SP) so the chain starts as early as possible.
  * pred_obj background SmoothL1 sum streams once as [128,2400] bf16:
    sum sl1 = 0.5*(QS - RD); QS = ACT Square+accum of x, RD = ACT
    Square+accum of r where DVE computes r = max(|x|,1)-1 in 3 bf16 ops.
  * All gt-derived per-positive planes (corners, centers, areas, fac*win)
    are host-precomputed and shipped with the bit-cast int32 offsets in
    one small input DMA.
  * ACT program order: warm-Exp (pins the exp/tanh/square table), QS,
    Exp, Tanh, RD, Exp(e4), then the single Ln last (one table switch,
    off the DVE critical path).  cls-loss is ACT Ln with accum_out;
    padded cls offsets hit a host-written 1.0 sentinel so ln(pad)=0.
  * Host combines per-core partial sums (weighted means).
"""
import numpy as np

B, A, C, H, W = 32, 3, 20, 160, 160
HW = H * W
M = 8            # cores
Bm = B // M      # images per core
SZ_OBJ = Bm * A * HW          # 307200
REC = 25                      # [d0,d1,d2,d3,obj,cls0..cls19] per cell
NTOT = Bm * A * HW * REC      # 7680000
NF = 17                       # host-data f32 planes (in units of K cols)
FCOLS = SZ_OBJ // 128         # 2400 free cols of the streamed obj tile

_NC_CACHE = {}


def _build_nc(K):
    import concourse.bass as bass
    import concourse.bacc as bacc
    import concourse.tile as tile
    from concourse import mybir

    f32 = mybir.dt.float32
    bf16 = mybir.dt.bfloat16
    i32 = mybir.dt.int32
    op = mybir.AluOpType
    act = mybir.ActivationFunctionType

    nc = bacc.Bacc("TRN2", target_bir_lowering=False, debug=False)
    preds_p = nc.dram_tensor("preds", [NTOT, 1], f32, kind="ExternalInput")
    obj16_p = nc.dram_tensor("obj16", [128, FCOLS], bf16, kind="ExternalInput")
    HCOLS = (NF + 20 + 1)  # planes | cls one-hot mask (20K) | rec offsets
    hdio_p = nc.dram_tensor("hdio", [128, HCOLS * K], f32,
                            kind="ExternalInput")
    NH = 2 if K > 1 else 1
    NCOLS = 2 + 4 * NH  # QS, RD, then per half: jk1, jkA, jkB, cls
    out_p = nc.dram_tensor("partials", [128, NCOLS], f32, kind="ExternalOutput")

    with tile.TileContext(nc) as tc, \
         tc.tile_pool(name="io", bufs=1) as io, \
         tc.tile_pool(name="wk", bufs=1) as wk:
        hdio = io.tile([128, HCOLS * K], f32)
        x16 = io.tile([128, FCOLS], bf16)
        r16 = io.tile([128, FCOLS], bf16)
        ngx = io.tile([128, FCOLS], bf16)
        ajunk = io.tile([128, FCOLS], bf16)   # ACT square scratch
        partials = io.tile([128, NCOLS], f32)

        # ---- input DMAs: hdio from DVE's queue (its compute starts late),
        # x16 from SP; the gathers own the Pool engine from t=0.
        nc.scalar.dma_start(out=hdio[:, :], in_=hdio_p[:, :])
        nc.sync.dma_start(out=x16[:, :], in_=obj16_p[:, :])
        ioffs = hdio[:, (NF + 20) * K:HCOLS * K].bitcast(i32)
        dv = wk.tile([128, K, REC], f32, tag="dv")
        for i_ in range(K):
            nc.gpsimd.indirect_dma_start(
                out=dv[:, i_, :], out_offset=None, in_=preds_p[:, :],
                in_offset=bass.IndirectOffsetOnAxis(
                    ap=ioffs[:, i_:i_ + 1], axis=0))

        # ---- split positives into two K-group halves: half A's math
        # starts right after its 2 gathers while half B still gathers.
        K0 = (K + 1) // 2
        halves = [(0, K0)]
        if K - K0 > 0:
            halves.append((K0, K - K0))

        tt = nc.vector.tensor_tensor
        ts = nc.vector.tensor_scalar
        stt = nc.vector.scalar_tensor_tensor
        A_ = nc.scalar.activation

        import itertools
        _cnt = itertools.count()

        hbase = hdio[:, :]

        # ---- ACT queue: warm-Exp pins the exp/tanh/square table at t=0.
        warm = wk.tile([128, 1], f32, name="warm", tag="warm")
        nc.vector.memset(warm[:, :], 1.0)
        warm2 = wk.tile([128, 1], f32, name="warm2", tag="warm2")
        A_(out=warm2[:, :], in_=warm[:, :], func=act.Exp)
        A_(out=ajunk[:, :], in_=x16[:, :], func=act.Square,
           accum_out=partials[:, 0:1])

        # ---- DVE stream work: r = max(|x|,1)-1 (bf16; ts ops in 4x mode)
        nc.vector.tensor_scalar(out=ngx[:, :], in0=x16[:, :],
                                scalar1=-1.0, scalar2=None, op0=op.mult)
        nc.vector.tensor_tensor(out=r16[:, :], in0=x16[:, :],
                                in1=ngx[:, :], op=op.max)
        nc.vector.tensor_scalar(out=r16[:, :], in0=r16[:, :],
                                scalar1=1.0, scalar2=1.0,
                                op0=op.max, op1=op.subtract)
        A_(out=ajunk[:, :], in_=r16[:, :], func=act.Square,
           accum_out=partials[:, 1:2])

        dvf = dv[:, :, :]
        pstep = dvf.ap[0]
        hstep = hbase.ap[0]

        for hidx, (g0, Kh) in enumerate(halves):
            pc0 = 2 + 4 * hidx

            def pr(n=2):
                nm = f"pr{next(_cnt)}"
                return wk.tile([128, n * Kh], f32, name=nm, tag=nm)

            def lo(t):
                return t[:, 0:Kh]

            def hi(t):
                return t[:, Kh:2 * Kh]

            def t2v(t):
                a = t[:]
                return bass.AP(tensor=a.tensor, offset=a.offset,
                               ap=[a.ap[0], [Kh, 2], [1, Kh]])

            def hpv(i):
                return bass.AP(tensor=hbase.tensor,
                               offset=hbase.offset + i * K + g0,
                               ap=[hstep, [K, 2], [1, Kh]])

            def hp1(i):
                return hdio[:, i * K + g0:i * K + g0 + Kh]

            d01 = bass.AP(tensor=dvf.tensor, offset=dvf.offset + g0 * REC,
                          ap=[pstep, [1, 2], [REC, Kh]])
            d23 = bass.AP(tensor=dvf.tensor, offset=dvf.offset + g0 * REC + 2,
                          ap=[pstep, [1, 2], [REC, Kh]])
            po = bass.AP(tensor=dvf.tensor, offset=dvf.offset + g0 * REC + 4,
                         ap=[pstep, [REC, Kh]])
            cls20 = bass.AP(tensor=dvf.tensor,
                            offset=dvf.offset + g0 * REC + 5,
                            ap=[pstep, [REC, Kh], [1, 20]])
            clsmask = bass.AP(tensor=hbase.tensor,
                              offset=hbase.offset + NF * K + g0 * 20,
                              ap=[hstep, [20, Kh], [1, 20]])

            ex = pr(); A_(out=t2v(ex), in_=d23, func=act.Exp)
            th = pr(); A_(out=t2v(th), in_=d01, func=act.Tanh)

            # cls extraction: one-hot mask over the 20 gathered class
            # values, reduce, +padflag so padded lanes feed Ln(1) = 0.
            mm = wk.tile([128, 20 * Kh], f32, tag=f"mm{hidx}")
            mmv = bass.AP(tensor=mm[:, :].tensor, offset=mm[:, :].offset,
                          ap=[mm[:, :].ap[0], [20, Kh], [1, 20]])
            nc.vector.tensor_tensor(out=mmv, in0=cls20, in1=clsmask,
                                    op=op.mult)
            pcgr = wk.tile([128, Kh], f32, tag=f"pcgr{hidx}")
            nc.vector.tensor_reduce(out=pcgr[:, :], in_=mmv,
                                    axis=mybir.AxisListType.X, op=op.add)
            pcg = wk.tile([128, Kh], f32, tag=f"pcg{hidx}")
            nc.vector.tensor_tensor(out=pcg[:, :], in0=pcgr[:, :],
                                    in1=hp1(16), op=op.add)

            # per-positive box math ([128, 2Kh]: x plane | y plane)
            c1 = pr(); tt(out=t2v(c1), in0=t2v(th), in1=hpv(0), op=op.add)
            wh1 = pr(); tt(out=t2v(wh1), in0=t2v(ex), in1=hpv(2), op=op.mult)
            b1lo = pr(); stt(out=b1lo[:], in0=wh1[:], scalar=-0.5, in1=c1[:],
                             op0=op.mult, op1=op.add)
            b1hi = pr(); stt(out=b1hi[:], in0=wh1[:], scalar=0.5, in1=c1[:],
                             op0=op.mult, op1=op.add)
            mnhi = pr(); tt(out=t2v(mnhi), in0=t2v(b1hi), in1=hpv(6),
                            op=op.min)
            mxlo = pr(); tt(out=t2v(mxlo), in0=t2v(b1lo), in1=hpv(4),
                            op=op.max)
            itax = pr(); tt(out=itax[:], in0=mnhi[:], in1=mxlo[:],
                            op=op.subtract)
            itax2 = pr(); ts(out=itax2[:], in0=itax[:], scalar1=0.0,
                             scalar2=None, op0=op.max)
            inter = pr(1); tt(out=inter[:], in0=lo(itax2), in1=hi(itax2),
                              op=op.mult)
            area1 = pr(1); tt(out=area1[:], in0=lo(wh1), in1=hi(wh1),
                              op=op.mult)
            u1 = pr(1); tt(out=u1[:], in0=area1[:], in1=hp1(12), op=op.add)
            u2 = pr(1); stt(out=u2[:], in0=inter[:], scalar=-1.0, in1=u1[:],
                            op0=op.mult, op1=op.add)
            ru = pr(1); nc.vector.reciprocal(out=ru[:], in_=u2[:])
            iou = pr(1); tt(out=iou[:], in0=inter[:], in1=ru[:], op=op.mult)
            # center-distance / angle terms  (s = gt_center - pred_center)
            cwmax = pr(); tt(out=t2v(cwmax), in0=t2v(b1hi), in1=hpv(6),
                             op=op.max)
            cwmin = pr(); tt(out=t2v(cwmin), in0=t2v(b1lo), in1=hpv(4),
                             op=op.min)
            cw = pr(); tt(out=cw[:], in0=cwmax[:], in1=cwmin[:],
                          op=op.subtract)
            s = pr(); tt(out=t2v(s), in0=hpv(10), in1=t2v(c1),
                         op=op.subtract)
            rcw = pr(); nc.vector.reciprocal(out=rcw[:], in_=cw[:])
            srw = pr(); tt(out=srw[:], in0=s[:], in1=rcw[:], op=op.mult)
            rho = pr(); tt(out=rho[:], in0=srw[:], in1=srw[:], op=op.mult)
            sqs = pr(); tt(out=sqs[:], in0=s[:], in1=s[:], op=op.mult)
            sig2 = pr(1); tt(out=sig2[:], in0=lo(sqs), in1=hi(sqs),
                             op=op.add)
            prod = pr(1); tt(out=prod[:], in0=lo(s), in1=hi(s), op=op.mult)
            aprod = pr(1); stt(out=aprod[:], in0=prod[:], scalar=-1.0,
                               in1=prod[:], op0=op.mult, op1=op.max)
            am2 = pr(1); tt(out=am2[:], in0=aprod[:], in1=sig2[:],
                            op=op.subtract)
            rsig2 = pr(1); nc.vector.reciprocal(out=rsig2[:], in_=sig2[:])
            gamma = pr(1); stt(out=gamma[:], in0=am2[:], scalar=2.0,
                               in1=rsig2[:], op0=op.mult, op1=op.mult)
            # shape-cost omiga terms
            wd = pr(); tt(out=t2v(wd), in0=t2v(wh1), in1=hpv(8),
                          op=op.subtract)
            wda = pr(); stt(out=wda[:], in0=wd[:], scalar=-1.0, in1=wd[:],
                            op0=op.mult, op1=op.max)
            mxw = pr(); tt(out=t2v(mxw), in0=t2v(wh1), in1=hpv(8), op=op.max)
            rmx = pr(); nc.vector.reciprocal(out=rmx[:], in_=mxw[:])
            g4 = wk.tile([128, 4 * Kh], f32, tag=f"g4{hidx}")
            tt(out=g4[:, 0:Kh], in0=gamma[:], in1=rho[:, 0:Kh], op=op.mult)
            tt(out=g4[:, Kh:2 * Kh], in0=gamma[:], in1=rho[:, Kh:2 * Kh],
               op=op.mult)
            stt(out=g4[:, 2 * Kh:4 * Kh], in0=wda[:], scalar=-1.0,
                in1=rmx[:], op0=op.mult, op1=op.mult)
            e4 = wk.tile([128, 4 * Kh], f32, tag=f"e4{hidx}")
            A_(out=e4[:, :], in_=g4[:, :], func=act.Exp)
            egs = pr(1); tt(out=egs[:], in0=e4[:, 0:Kh], in1=e4[:, Kh:2 * Kh],
                            op=op.add)
            oneo = pr(); ts(out=oneo[:], in0=e4[:, 2 * Kh:4 * Kh],
                            scalar1=-1.0, scalar2=1.0, op0=op.mult,
                            op1=op.add)
            sq1 = pr(); tt(out=sq1[:], in0=oneo[:], in1=oneo[:], op=op.mult)
            sh = pr(); tt(out=sh[:], in0=sq1[:], in1=sq1[:], op=op.mult)
            shs = pr(1); tt(out=shs[:], in0=lo(sh), in1=hi(sh), op=op.add)
            # q = 0.5*(shs-egs) - iou ; 1-siou = q+2 ; siou = -q-1
            p_ = pr(1); tt(out=p_[:], in0=shs[:], in1=egs[:], op=op.subtract)
            q_ = pr(1); stt(out=q_[:], in0=p_[:], scalar=0.5, in1=iou[:],
                            op0=op.mult, op1=op.subtract)
            jnk1 = pr(1); stt(out=jnk1[:], in0=q_[:], scalar=1.0, in1=hp1(15),
                              op0=op.mult, op1=op.mult,
                              accum_out=partials[:, pc0:pc0 + 1])
            # obj correction at positive cells:
            # sum fac*win*sl1(po - siou)  -  sum 0.75*win*sl1(po)
            dif = pr(1); stt(out=dif[:], in0=q_[:], scalar=1.0, in1=po,
                             op0=op.add, op1=op.add)
            ad = pr(1); stt(out=ad[:], in0=dif[:], scalar=-1.0, in1=dif[:],
                            op0=op.mult, op1=op.max)
            md = pr(1); ts(out=md[:], in0=ad[:], scalar1=1.0, scalar2=None,
                           op0=op.min)
            ud = pr(1); stt(out=ud[:], in0=md[:], scalar=-0.5, in1=ad[:],
                            op0=op.mult, op1=op.add)
            sdA = pr(1); tt(out=sdA[:], in0=md[:], in1=ud[:], op=op.mult)
            jnkA = pr(1); stt(out=jnkA[:], in0=sdA[:], scalar=1.0,
                              in1=hp1(13), op0=op.mult, op1=op.mult,
                              accum_out=partials[:, pc0 + 1:pc0 + 2])
            a2 = pr(1); stt(out=a2[:], in0=po, scalar=-1.0, in1=po,
                            op0=op.mult, op1=op.max)
            m2 = pr(1); ts(out=m2[:], in0=a2[:], scalar1=1.0, scalar2=None,
                           op0=op.min)
            u2t = pr(1); stt(out=u2t[:], in0=m2[:], scalar=-0.5, in1=a2[:],
                             op0=op.mult, op1=op.add)
            sdB = pr(1); tt(out=sdB[:], in0=m2[:], in1=u2t[:], op=op.mult)
            jnkB = pr(1); stt(out=jnkB[:], in0=sdB[:], scalar=1.0,
                              in1=hp1(14), op0=op.mult, op1=op.mult,
                              accum_out=partials[:, pc0 + 2:pc0 + 3])
            # cls loss: the single Ln per half, forced late on ACT via a
            # sim-time floor so the scheduler keeps it after the Exps.
            lnp = pr(1)
            with tc.tile_wait_until(0.05 + 0.002 * hidx):
                A_(out=lnp[:], in_=pcg[:, :], func=act.Ln,
                   accum_out=partials[:, pc0 + 3:pc0 + 4])

        nc.sync.dma_start(out=out_p[:, :], in_=partials[:, :])

    return nc


def _get_nc(K, finalized=True):
    key = (K, finalized)
    if key not in _NC_CACHE:
        nc = _build_nc(K)
        if finalized:
            nc.finalize()
        else:
            nc.compile()
        _NC_CACHE[key] = nc
    return _NC_CACHE[key]


def _pack(vals, K, fill, dtype):
    """lane j = i*128 + p  ->  tile[p, i]."""
    out = np.full((K, 128), fill, dtype)
    out.reshape(-1)[:len(vals)] = vals
    return out.T


def host_prep(pred_obj, pred_delta_box, pred_cls, gt_box, gt_cls,
              p_batch_idx, p_x_idx, p_y_idx, p_anchor_idx, anchors):
    """Shard inputs; all-integer index prep. Returns (in_maps, K, P)."""
    from concourse import mybir
    f32 = np.float32
    bf16 = mybir.dt.np(mybir.dt.bfloat16)
    pred_obj = np.asarray(pred_obj, f32)
    pred_delta_box = np.asarray(pred_delta_box, f32)
    pred_cls = np.asarray(pred_cls, f32)
    gt_box = np.asarray(gt_box, f32)
    gt_cls = np.asarray(gt_cls, np.int64)
    p_b = np.asarray(p_batch_idx, np.int64)
    p_x = np.asarray(p_x_idx, np.int64)
    p_y = np.asarray(p_y_idx, np.int64)
    p_a = np.asarray(p_anchor_idx, np.int64)
    anchors = np.asarray(anchors, f32)
    P = len(p_b)

    n_img = np.bincount(p_b, minlength=B)
    # duplicate (b,y,x,a) cells: last occurrence wins (matches XLA scatter)
    cell = ((p_b * H + p_y) * W + p_x) * A + p_a
    win = np.zeros(P, f32)
    _, ridx = np.unique(cell[::-1], return_index=True)
    win[P - 1 - ridx] = 1.0

    core_of = p_b // Bm
    counts = np.bincount(core_of, minlength=M)
    Pmax = int(counts.max())
    K = max(1, -(-Pmax // 128))

    in_maps = []
    for m in range(M):
        sel = core_of == m
        bl = p_b[sel] - m * Bm
        xj, yj, aj, cj = p_x[sel], p_y[sel], p_a[sel], gt_cls[sel]
        base = bl * A + aj
        sp = yj * W + xj
        off_rec = (base * HW + sp) * REC

        i32 = np.int32
        ioffs = _pack(off_rec, K, 0, i32)

        gtb = gt_box[sel]
        ancg = anchors[aj]
        winm = win[sel]
        fac = (0.25 * HW / n_img[p_b[sel]]).astype(f32)
        hd_planes = [
            _pack(xj.astype(f32), K, 0.0, f32),
            _pack(yj.astype(f32), K, 0.0, f32),
            _pack(ancg[:, 0] * W, K, 16.0, f32),
            _pack(ancg[:, 1] * H, K, 16.0, f32),
            _pack(gtb[:, 0] - 0.5 * gtb[:, 2], K, 40.0, f32),   # b2lo x
            _pack(gtb[:, 1] - 0.5 * gtb[:, 3], K, 40.0, f32),   # b2lo y
            _pack(gtb[:, 0] + 0.5 * gtb[:, 2], K, 120.0, f32),  # b2hi x
            _pack(gtb[:, 1] + 0.5 * gtb[:, 3], K, 120.0, f32),  # b2hi y
            _pack(gtb[:, 2], K, 80.0, f32),                     # gt w
            _pack(gtb[:, 3], K, 80.0, f32),                     # gt h
            _pack(gtb[:, 0], K, 80.0, f32),                     # gt cx
            _pack(gtb[:, 1], K, 80.0, f32),                     # gt cy
            _pack(gtb[:, 2] * gtb[:, 3], K, 6400.0, f32),       # area2
            _pack(fac * winm, K, 0.0, f32),                     # fac*win
            _pack(0.75 * winm, K, 0.0, f32),                    # 0.75*win
            _pack(np.ones(int(sel.sum()), f32), K, 0.0, f32),   # valid
            _pack(np.zeros(int(sel.sum()), f32), K, 1.0, f32),  # padflag
        ]
        nsel = int(sel.sum())
        onehot = np.zeros((nsel, 20), f32)
        onehot[np.arange(nsel), cj] = 1.0
        mask_planes = [
            _pack(np.ascontiguousarray(onehot[:, c]), K, 0.0, f32)
            for c in range(20)
        ]
        # mask layout in hdio cols: [c*K + k] per class-c plane -> the
        # device view [(20,K),(1,20)] reads col 20*k + c, so interleave.
        maskm = np.stack(mask_planes, axis=2).reshape(128, 20 * K, order='C')
        hdio = np.concatenate(hd_planes + [maskm, ioffs.view(f32)], axis=1)

        preds = np.empty(NTOT, f32)
        rec = preds.reshape(Bm, A, H, W, REC)
        objm = pred_obj[m * Bm:(m + 1) * Bm]
        rec[..., 0:4] = pred_delta_box[m * Bm:(m + 1) * Bm].transpose(0, 1, 3, 4, 2)
        rec[..., 4] = objm
        rec[..., 5:] = pred_cls[m * Bm:(m + 1) * Bm].transpose(0, 1, 3, 4, 2)

        obj16 = objm.reshape(-1).astype(bf16).reshape(128, FCOLS)

        in_maps.append({
            "preds": preds.reshape(NTOT, 1),
            "obj16": obj16,
            "hdio": np.ascontiguousarray(hdio),
        })
    return in_maps, K, P


def combine(partials_list, P):
    """Host reduction of per-core [128, 2+4*nh] partial sums."""
    ncols = np.asarray(partials_list[0]).shape[1]
    tot = np.zeros(ncols, np.float64)
    for pt in partials_list:
        tot += np.asarray(pt, np.float64).sum(axis=0)
    QS, RD = tot[0], tot[1]
    nh = (ncols - 2) // 4
    jk1 = sum(tot[2 + 4 * h] for h in range(nh))
    jkA = sum(tot[3 + 4 * h] for h in range(nh))
    jkB = sum(tot[4 + 4 * h] for h in range(nh))
    lnsum = sum(tot[5 + 4 * h] for h in range(nh))
    iou_loss = (jk1 + 2.0 * P) / P
    cls_loss = -lnsum / P
    obj_loss = (0.375 * (QS - RD) + (jkA - jkB)) / (B * A * H * W)
    tot_loss = iou_loss + 4 * obj_loss + 2 * cls_loss
    return (np.float32(iou_loss), np.float32(obj_loss),
            np.float32(cls_loss), np.float32(tot_loss))


def kernel(pred_obj, pred_delta_box, pred_cls, gt_box, gt_cls,
           p_batch_idx, p_x_idx, p_y_idx, p_anchor_idx, anchors):
    from concourse.bass_utils import run_bass_kernel_spmd
    in_maps, K, P = host_prep(pred_obj, pred_delta_box, pred_cls, gt_box,
                              gt_cls, p_batch_idx, p_x_idx, p_y_idx,
                              p_anchor_idx, anchors)
    nc = _get_nc(K)
    res = run_bass_kernel_spmd(nc, in_maps, list(range(M))).results
    return combine([r["partials"] for r in res], P)


# revision 14
# speedup vs baseline: 1.1940x; 1.1940x over previous
"""DetectorLoss on 8 Trainium2 NeuronCores (Bass/Tile).

Strategy (data-parallel over batch, 4 images per core):
  * Sparse positive-cell reads use the HW indirect-DMA shape (ONE offset
    per partition, contiguous block): the host interleaves
    [d0,d1,d2,d3,obj] per cell so 4 gathers cover box deltas AND obj;
    4 more cover cls.  Gathers run on a clean Pool queue (input DMAs are
    dispatched from SP) so the chain starts as early as possible.
  * pred_obj background SmoothL1 sum streams once as [128,2400] bf16:
    sum sl1 = 0.5*(QS - RD); QS = ACT Square+accum of x, RD = ACT
    Square+accum of r where DVE computes r = max(|x|,1)-1 in 3 bf16 ops.
  * All gt-derived per-positive planes (corners, centers, areas, fac*win)
    are host-precomputed and shipped with the bit-cast int32 offsets in
    one small input DMA.
  * ACT program order: warm-Exp (pins the exp/tanh/square table), QS,
    Exp, Tanh, RD, Exp(e4), then the single Ln last (one table switch,
    off the DVE critical path).  cls-loss is ACT Ln with accum_out;
    padded cls offsets hit a host-written 1.0 sentinel so ln(pad)=0.
  * Host combines per-core partial sums (weighted means).
"""
import numpy as np

B, A, C, H, W = 32, 3, 20, 160, 160
HW = H * W
M = 8            # cores
Bm = B // M      # images per core
SZ_OBJ = Bm * A * HW          # 307200
REC = 25                      # [d0,d1,d2,d3,obj,cls0..cls19] per cell
NTOT = Bm * A * HW * REC      # 7680000
NF = 17                       # host-data f32 planes (in units of K cols)
FCOLS = SZ_OBJ // 128         # 2400 free cols of the streamed obj tile

_NC_CACHE = {}


def _build_nc(K):
    import concourse.bass as bass
    import concourse.bacc as bacc
    import concourse.tile as tile
    from concourse import mybir

    f32 = mybir.dt.float32
    bf16 = mybir.dt.bfloat16
    i32 = mybir.dt.int32
    op = mybir.AluOpType
    act = mybir.ActivationFunctionType

    nc = bacc.Bacc("TRN2", target_bir_lowering=False, debug=False)
    preds_p = nc.dram_tensor("preds", [NTOT, 1], f32, kind="ExternalInput")
    obj16_p = nc.dram_tensor("obj16", [128, FCOLS], bf16, kind="ExternalInput")
    HCOLS = (NF + 20 + 1)  # planes | cls one-hot mask (20K) | rec offsets
    hdio_p = nc.dram_tensor("hdio", [128, HCOLS * K], f32,
                            kind="ExternalInput")
    NCOLS = 6  # QS, RD, jk1(iou), jkA, jkB, cls
    out_p = nc.dram_tensor("partials", [128, NCOLS], f32, kind="ExternalOutput")

    with tile.TileContext(nc) as tc, \
         tc.tile_pool(name="io", bufs=1) as io, \
         tc.tile_pool(name="wk", bufs=1) as wk:
        hdio = io.tile([128, HCOLS * K], f32)
        x16 = io.tile([128, FCOLS], bf16)
        r16 = io.tile([128, FCOLS], bf16)
        ngx = io.tile([128, FCOLS], bf16)
        ajunk = io.tile([128, FCOLS], bf16)   # ACT square scratch
        partials = io.tile([128, NCOLS], f32)

        # ---- input DMAs: hdio from DVE's queue (its compute starts late),
        # x16 from SP; the gathers own the Pool engine from t=0.
        nc.scalar.dma_start(out=hdio[:, :], in_=hdio_p[:, :])
        nc.sync.dma_start(out=x16[:, :], in_=obj16_p[:, :])
        ioffs = hdio[:, (NF + 20) * K:HCOLS * K].bitcast(i32)
        dv = wk.tile([128, K, REC], f32, tag="dv")
        for i_ in range(K):
            nc.gpsimd.indirect_dma_start(
                out=dv[:, i_, :], out_offset=None, in_=preds_p[:, :],
                in_offset=bass.IndirectOffsetOnAxis(
                    ap=ioffs[:, i_:i_ + 1], axis=0))

        # ---- host-data plane views ----
        def hp(i, n=2):
            return hdio[:, i * K:(i + n) * K]

        pxy = hp(0)
        ancW = hp(2)
        b2lo = hp(4)
        b2hi = hp(6)
        gtwh = hp(8)
        gtc = hp(10)
        area2 = hp(12, 1)
        facw = hp(13, 1)
        w75 = hp(14, 1)
        valid = hp(15, 1)
        padf = hp(16, 1)
        clsmask = bass.AP(tensor=hdio[:, :].tensor,
                          offset=hdio[:, :].offset + NF * K,
                          ap=[hdio[:, :].ap[0], [20, K], [1, 20]])

        dvf = dv[:, :, :]
        d01 = bass.AP(tensor=dvf.tensor, offset=dvf.offset,
                      ap=[dvf.ap[0], [1, 2], [REC, K]])
        d23 = bass.AP(tensor=dvf.tensor, offset=dvf.offset + 2,
                      ap=[dvf.ap[0], [1, 2], [REC, K]])
        po = bass.AP(tensor=dvf.tensor, offset=dvf.offset + 4,
                     ap=[dvf.ap[0], [REC, K]])
        cls20 = bass.AP(tensor=dvf.tensor, offset=dvf.offset + 5,
                        ap=[dvf.ap[0], [REC, K], [1, 20]])

        tt = nc.vector.tensor_tensor
        ts = nc.vector.tensor_scalar
        stt = nc.vector.scalar_tensor_tensor
        A_ = nc.scalar.activation

        import itertools
        _cnt = itertools.count()

        def pr(n=2):
            nm = f"pr{next(_cnt)}"
            return wk.tile([128, n * K], f32, name=nm, tag=nm)

        def lo(t):
            return t[:, 0:K]

        def hi(t):
            return t[:, K:2 * K]

        def v3(t):
            a = t[:]
            return bass.AP(tensor=a.tensor, offset=a.offset,
                           ap=[a.ap[0], [K, 2], [1, K]])

        # ---- ACT queue: warm-Exp pins the exp/tanh/square table at t=0.
        warm = wk.tile([128, 1], f32, name="warm", tag="warm")
        nc.vector.memset(warm[:, :], 1.0)
        warm2 = wk.tile([128, 1], f32, name="warm2", tag="warm2")
        A_(out=warm2[:, :], in_=warm[:, :], func=act.Exp)
        A_(out=ajunk[:, :], in_=x16[:, :], func=act.Square,
           accum_out=partials[:, 0:1])
        ex = pr(); A_(out=v3(ex), in_=d23, func=act.Exp)
        th = pr(); A_(out=v3(th), in_=d01, func=act.Tanh)

        # ---- DVE stream work: r = max(|x|,1)-1 (bf16; ts ops in 4x mode)
        nc.vector.tensor_scalar(out=ngx[:, :], in0=x16[:, :],
                                scalar1=-1.0, scalar2=None, op0=op.mult)
        nc.vector.tensor_tensor(out=r16[:, :], in0=x16[:, :],
                                in1=ngx[:, :], op=op.max)
        nc.vector.tensor_scalar(out=r16[:, :], in0=r16[:, :],
                                scalar1=1.0, scalar2=1.0,
                                op0=op.max, op1=op.subtract)
        A_(out=ajunk[:, :], in_=r16[:, :], func=act.Square,
           accum_out=partials[:, 1:2])

        # ---- cls extraction: one-hot mask over the 20 gathered class
        # values, reduce, +padflag so padded lanes feed Ln(1) = 0.
        mm = wk.tile([128, 20 * K], f32, tag="mm")
        mmv = bass.AP(tensor=mm[:, :].tensor, offset=mm[:, :].offset,
                      ap=[mm[:, :].ap[0], [20, K], [1, 20]])
        nc.vector.tensor_tensor(out=mmv, in0=cls20, in1=clsmask, op=op.mult)
        pcgr = wk.tile([128, K], f32, tag="pcgr")
        nc.vector.tensor_reduce(out=pcgr[:, :],
                                in_=bass.AP(tensor=mm[:, :].tensor,
                                            offset=mm[:, :].offset,
                                            ap=[mm[:, :].ap[0], [20, K],
                                                [1, 20]]),
                                axis=mybir.AxisListType.X, op=op.add)
        pcg = wk.tile([128, K], f32, tag="pcg")
        nc.vector.tensor_tensor(out=pcg[:, :], in0=pcgr[:, :], in1=padf,
                                op=op.add)

        # ---- per-positive box math on DVE ([128, 2K]: x plane | y plane)
        c1 = pr(); tt(out=c1[:], in0=th[:], in1=pxy, op=op.add)
        wh1 = pr(); tt(out=wh1[:], in0=ex[:], in1=ancW, op=op.mult)
        b1lo = pr(); stt(out=b1lo[:], in0=wh1[:], scalar=-0.5, in1=c1[:],
                         op0=op.mult, op1=op.add)
        b1hi = pr(); stt(out=b1hi[:], in0=wh1[:], scalar=0.5, in1=c1[:],
                         op0=op.mult, op1=op.add)
        mnhi = pr(); tt(out=mnhi[:], in0=b1hi[:], in1=b2hi, op=op.min)
        mxlo = pr(); tt(out=mxlo[:], in0=b1lo[:], in1=b2lo, op=op.max)
        itax = pr(); tt(out=itax[:], in0=mnhi[:], in1=mxlo[:], op=op.subtract)
        itax2 = pr(); ts(out=itax2[:], in0=itax[:], scalar1=0.0, scalar2=None,
                         op0=op.max)
        inter = pr(1); tt(out=inter[:], in0=lo(itax2), in1=hi(itax2),
                          op=op.mult)
        area1 = pr(1); tt(out=area1[:], in0=lo(wh1), in1=hi(wh1), op=op.mult)
        u1 = pr(1); tt(out=u1[:], in0=area1[:], in1=area2, op=op.add)
        u2 = pr(1); stt(out=u2[:], in0=inter[:], scalar=-1.0, in1=u1[:],
                        op0=op.mult, op1=op.add)
        ru = pr(1); nc.vector.reciprocal(out=ru[:], in_=u2[:])
        iou = pr(1); tt(out=iou[:], in0=inter[:], in1=ru[:], op=op.mult)
        # center-distance / angle terms  (s = gt_center - pred_center)
        cwmax = pr(); tt(out=cwmax[:], in0=b1hi[:], in1=b2hi, op=op.max)
        cwmin = pr(); tt(out=cwmin[:], in0=b1lo[:], in1=b2lo, op=op.min)
        cw = pr(); tt(out=cw[:], in0=cwmax[:], in1=cwmin[:], op=op.subtract)
        s = pr(); tt(out=s[:], in0=gtc, in1=c1[:], op=op.subtract)
        rcw = pr(); nc.vector.reciprocal(out=rcw[:], in_=cw[:])
        srw = pr(); tt(out=srw[:], in0=s[:], in1=rcw[:], op=op.mult)
        rho = pr(); tt(out=rho[:], in0=srw[:], in1=srw[:], op=op.mult)
        sqs = pr(); tt(out=sqs[:], in0=s[:], in1=s[:], op=op.mult)
        sig2 = pr(1); tt(out=sig2[:], in0=lo(sqs), in1=hi(sqs), op=op.add)
        prod = pr(1); tt(out=prod[:], in0=lo(s), in1=hi(s), op=op.mult)
        aprod = pr(1); stt(out=aprod[:], in0=prod[:], scalar=-1.0,
                           in1=prod[:], op0=op.mult, op1=op.max)
        am2 = pr(1); tt(out=am2[:], in0=aprod[:], in1=sig2[:], op=op.subtract)
        rsig2 = pr(1); nc.vector.reciprocal(out=rsig2[:], in_=sig2[:])
        # gamma = angle_cost - 2 = 2*(|s_cw*s_ch| - sigma^2)/sigma^2
        gamma = pr(1); stt(out=gamma[:], in0=am2[:], scalar=2.0, in1=rsig2[:],
                           op0=op.mult, op1=op.mult)
        # shape-cost omiga terms
        wd = pr(); tt(out=wd[:], in0=wh1[:], in1=gtwh, op=op.subtract)
        wda = pr(); stt(out=wda[:], in0=wd[:], scalar=-1.0, in1=wd[:],
                        op0=op.mult, op1=op.max)
        mxw = pr(); tt(out=mxw[:], in0=wh1[:], in1=gtwh, op=op.max)
        rmx = pr(); nc.vector.reciprocal(out=rmx[:], in_=mxw[:])
        g4 = wk.tile([128, 4 * K], f32, tag="g4")
        tt(out=g4[:, 0:K], in0=gamma[:], in1=rho[:, 0:K], op=op.mult)
        tt(out=g4[:, K:2 * K], in0=gamma[:], in1=rho[:, K:2 * K], op=op.mult)
        stt(out=g4[:, 2 * K:4 * K], in0=wda[:], scalar=-1.0, in1=rmx[:],
            op0=op.mult, op1=op.mult)
        e4 = wk.tile([128, 4 * K], f32, tag="e4")
        A_(out=e4[:, :], in_=g4[:, :], func=act.Exp)
        egs = pr(1); tt(out=egs[:], in0=e4[:, 0:K], in1=e4[:, K:2 * K],
                        op=op.add)
        oneo = pr(); ts(out=oneo[:], in0=e4[:, 2 * K:4 * K], scalar1=-1.0,
                        scalar2=1.0, op0=op.mult, op1=op.add)
        sq1 = pr(); tt(out=sq1[:], in0=oneo[:], in1=oneo[:], op=op.mult)
        sh = pr(); tt(out=sh[:], in0=sq1[:], in1=sq1[:], op=op.mult)
        shs = pr(1); tt(out=shs[:], in0=lo(sh), in1=hi(sh), op=op.add)
        # q = 0.5*(shs-egs) - iou ; 1-siou = q+2 ; siou = -q-1
        p_ = pr(1); tt(out=p_[:], in0=shs[:], in1=egs[:], op=op.subtract)
        q_ = pr(1); stt(out=q_[:], in0=p_[:], scalar=0.5, in1=iou[:],
                        op0=op.mult, op1=op.subtract)
        jnk1 = pr(1); stt(out=jnk1[:], in0=q_[:], scalar=1.0, in1=valid,
                          op0=op.mult, op1=op.mult,
                          accum_out=partials[:, 2:3])
        # obj correction at positive cells:
        # sum fac*win*sl1(po - siou)  -  sum 0.75*win*sl1(po)
        dif = pr(1); stt(out=dif[:], in0=q_[:], scalar=1.0, in1=po,
                         op0=op.add, op1=op.add)
        ad = pr(1); stt(out=ad[:], in0=dif[:], scalar=-1.0, in1=dif[:],
                        op0=op.mult, op1=op.max)
        md = pr(1); ts(out=md[:], in0=ad[:], scalar1=1.0, scalar2=None,
                       op0=op.min)
        ud = pr(1); stt(out=ud[:], in0=md[:], scalar=-0.5, in1=ad[:],
                        op0=op.mult, op1=op.add)
        sdA = pr(1); tt(out=sdA[:], in0=md[:], in1=ud[:], op=op.mult)
        jnkA = pr(1); stt(out=jnkA[:], in0=sdA[:], scalar=1.0, in1=facw,
                          op0=op.mult, op1=op.mult,
                          accum_out=partials[:, 3:4])
        a2 = pr(1); stt(out=a2[:], in0=po, scalar=-1.0, in1=po,
                        op0=op.mult, op1=op.max)
        m2 = pr(1); ts(out=m2[:], in0=a2[:], scalar1=1.0, scalar2=None,
                       op0=op.min)
        u2t = pr(1); stt(out=u2t[:], in0=m2[:], scalar=-0.5, in1=a2[:],
                         op0=op.mult, op1=op.add)
        sdB = pr(1); tt(out=sdB[:], in0=m2[:], in1=u2t[:], op=op.mult)
        jnkB = pr(1); stt(out=jnkB[:], in0=sdB[:], scalar=1.0, in1=w75,
                          op0=op.mult, op1=op.mult,
                          accum_out=partials[:, 4:5])
        # cls loss: the single Ln, forced LAST on ACT via a sim-time floor
        # so the scheduler cannot slot it before e4 (which would cost an
        # extra act-table load).  Padded cls offsets hit the 1.0 sentinel
        # so ln(pad) = 0 and the accumulate needs no mask.
        lnp = pr(1)
        with tc.tile_wait_until(0.05):
            A_(out=lnp[:], in_=pcg[:, :], func=act.Ln,
               accum_out=partials[:, 5:6])

        nc.sync.dma_start(out=out_p[:, :], in_=partials[:, :])

    return nc


def _get_nc(K, finalized=True):
    key = (K, finalized)
    if key not in _NC_CACHE:
        nc = _build_nc(K)
        if finalized:
            nc.finalize()
        else:
            nc.compile()
        _NC_CACHE[key] = nc
    return _NC_CACHE[key]


def _pack(vals, K, fill, dtype):
    """lane j = i*128 + p  ->  tile[p, i]."""
    out = np.full((K, 128), fill, dtype)
    out.reshape(-1)[:len(vals)] = vals
    return out.T


def host_prep(pred_obj, pred_delta_box, pred_cls, gt_box, gt_cls,
              p_batch_idx, p_x_idx, p_y_idx, p_anchor_idx, anchors):
    """Shard inputs; all-integer index prep. Returns (in_maps, K, P)."""
    from concourse import mybir
    f32 = np.float32
    bf16 = mybir.dt.np(mybir.dt.bfloat16)
    pred_obj = np.asarray(pred_obj, f32)
    pred_delta_box = np.asarray(pred_delta_box, f32)
    pred_cls = np.asarray(pred_cls, f32)
    gt_box = np.asarray(gt_box, f32)
    gt_cls = np.asarray(gt_cls, np.int64)
    p_b = np.asarray(p_batch_idx, np.int64)
    p_x = np.asarray(p_x_idx, np.int64)
    p_y = np.asarray(p_y_idx, np.int64)
    p_a = np.asarray(p_anchor_idx, np.int64)
    anchors = np.asarray(anchors, f32)
    P = len(p_b)

    n_img = np.bincount(p_b, minlength=B)
    # duplicate (b,y,x,a) cells: last occurrence wins (matches XLA scatter)
    cell = ((p_b * H + p_y) * W + p_x) * A + p_a
    win = np.zeros(P, f32)
    _, ridx = np.unique(cell[::-1], return_index=True)
    win[P - 1 - ridx] = 1.0

    core_of = p_b // Bm
    counts = np.bincount(core_of, minlength=M)
    Pmax = int(counts.max())
    K = max(1, -(-Pmax // 128))

    in_maps = []
    for m in range(M):
        sel = core_of == m
        bl = p_b[sel] - m * Bm
        xj, yj, aj, cj = p_x[sel], p_y[sel], p_a[sel], gt_cls[sel]
        base = bl * A + aj
        sp = yj * W + xj
        off_rec = (base * HW + sp) * REC

        i32 = np.int32
        ioffs = _pack(off_rec, K, 0, i32)

        gtb = gt_box[sel]
        ancg = anchors[aj]
        winm = win[sel]
        fac = (0.25 * HW / n_img[p_b[sel]]).astype(f32)
        hd_planes = [
            _pack(xj.astype(f32), K, 0.0, f32),
            _pack(yj.astype(f32), K, 0.0, f32),
            _pack(ancg[:, 0] * W, K, 16.0, f32),
            _pack(ancg[:, 1] * H, K, 16.0, f32),
            _pack(gtb[:, 0] - 0.5 * gtb[:, 2], K, 40.0, f32),   # b2lo x
            _pack(gtb[:, 1] - 0.5 * gtb[:, 3], K, 40.0, f32),   # b2lo y
            _pack(gtb[:, 0] + 0.5 * gtb[:, 2], K, 120.0, f32),  # b2hi x
            _pack(gtb[:, 1] + 0.5 * gtb[:, 3], K, 120.0, f32),  # b2hi y
            _pack(gtb[:, 2], K, 80.0, f32),                     # gt w
            _pack(gtb[:, 3], K, 80.0, f32),                     # gt h
            _pack(gtb[:, 0], K, 80.0, f32),                     # gt cx
            _pack(gtb[:, 1], K, 80.0, f32),                     # gt cy
            _pack(gtb[:, 2] * gtb[:, 3], K, 6400.0, f32),       # area2
            _pack(fac * winm, K, 0.0, f32),                     # fac*win
            _pack(0.75 * winm, K, 0.0, f32),                    # 0.75*win
            _pack(np.ones(int(sel.sum()), f32), K, 0.0, f32),   # valid
            _pack(np.zeros(int(sel.sum()), f32), K, 1.0, f32),  # padflag
        ]
        nsel = int(sel.sum())
        onehot = np.zeros((nsel, 20), f32)
        onehot[np.arange(nsel), cj] = 1.0
        mask_planes = [
            _pack(np.ascontiguousarray(onehot[:, c]), K, 0.0, f32)
            for c in range(20)
        ]
        # mask layout in hdio cols: [c*K + k] per class-c plane -> the
        # device view [(20,K),(1,20)] reads col 20*k + c, so interleave.
        maskm = np.stack(mask_planes, axis=2).reshape(128, 20 * K, order='C')
        hdio = np.concatenate(hd_planes + [maskm, ioffs.view(f32)], axis=1)

        preds = np.empty(NTOT, f32)
        rec = preds.reshape(Bm, A, H, W, REC)
        objm = pred_obj[m * Bm:(m + 1) * Bm]
        rec[..., 0:4] = pred_delta_box[m * Bm:(m + 1) * Bm].transpose(0, 1, 3, 4, 2)
        rec[..., 4] = objm
        rec[..., 5:] = pred_cls[m * Bm:(m + 1) * Bm].transpose(0, 1, 3, 4, 2)

        obj16 = objm.reshape(-1).astype(bf16).reshape(128, FCOLS)

        in_maps.append({
            "preds": preds.reshape(NTOT, 1),
            "obj16": obj16,
            "hdio": np.ascontiguousarray(hdio),
        })
    return in_maps, K, P


def combine(partials_list, P):
    """Host reduction of per-core [128, 6] partial sums."""
    tot = np.zeros(6, np.float64)
    for pt in partials_list:
        tot += np.asarray(pt, np.float64).sum(axis=0)
    QS, RD, jk1, jkA, jkB, lnsum = tot
    iou_loss = (jk1 + 2.0 * P) / P
    cls_loss = -lnsum / P
    obj_loss = (0.375 * (QS - RD) + (jkA - jkB)) / (B * A * H * W)
    tot_loss = iou_loss + 4 * obj_loss + 2 * cls_loss
    return (np.float32(iou_loss), np.float32(obj_loss),
            np.float32(cls_loss), np.float32(tot_loss))


def kernel(pred_obj, pred_delta_box, pred_cls, gt_box, gt_cls,
           p_batch_idx, p_x_idx, p_y_idx, p_anchor_idx, anchors):
    from concourse.bass_utils import run_bass_kernel_spmd
    in_maps, K, P = host_prep(pred_obj, pred_delta_box, pred_cls, gt_box,
                              gt_cls, p_batch_idx, p_x_idx, p_y_idx,
                              p_anchor_idx, anchors)
    nc = _get_nc(K)
    res = run_bass_kernel_spmd(nc, in_maps, list(range(M))).results
    return combine([r["partials"] for r in res], P)
